# revision 1
# baseline (speedup 1.0000x reference)
"""DiffusionGCN (2-layer GCN + linear head) on 8 Trainium2 NeuronCores.

Strategy (graph/data parallel, per sharding hint):
  - Nodes sharded across 8 cores (12544 padded nodes each); edges partitioned
    by destination core and grouped by destination supertile (256 nodes).
  - Weights replicated; gathered source features fetched per-edge from a
    full-node feature table in HBM via indirect DMA (128 rows / instruction,
    one row per SBUF partition).
  - Symmetric-norm trick: out[v] = dis[v] * sum_{e: dst=v} (dis[src] * h[src]),
    so the source-side scale is folded into the gather table (htilde = dis*h)
    and the dest-side scale is applied after aggregation. The linear transform
    commutes with the aggregation, so W is applied AFTER the segment-sum on the
    core's own 12544-node shard only.
  - Segment-sum on device via one-hot matmuls: for each 128-edge block,
    onehot[e, j] = (dstlocal[e] == j) over a 256-wide supertile, built with a
    single DVE is_equal; PE accumulates msg^T @ onehot into PSUM per supertile.
  - 3 SPMD launches: prep (deg -> dis, htilde0 = dis*x), layer 1, layer 2 +
    classifier head. The host only re-shards between launches (concat/split).
"""

import os
import sys
from contextlib import ExitStack

import numpy as np

for _p in ("/opt/trn_rl_repo", "/root/.axon_site/_ro/trn_rl_repo"):
    if os.path.isdir(_p) and _p not in sys.path:
        sys.path.insert(0, _p)

import concourse.bacc as bacc
import concourse.bass as bass
import concourse.mybir as mybir
import concourse.tile as tile
from concourse.bass_utils import run_bass_kernel_spmd

F32 = mybir.dt.float32
F32R = mybir.dt.float32r
I32 = mybir.dt.int32

N = 100000
E = 1600000
D = 128
H = 128
C = 64
NCORES = 8
NPAD = 100352            # 8 * 12544
NV = NPAD // NCORES      # 12544 nodes per core
NT = NV // 128           # 98 tiles per core
NST = NT // 2            # 49 supertiles (256 nodes) per core
USE_F32R = True


MMDT = F32R if USE_F32R else F32  # dtype of scatter-matmul operands


# ----------------------------------------------------------------- host prep

def _prep_graph(edge_index):
    """Partition/sort edges; build per-core gather-index and dstlocal arrays
    with a block structure that is IDENTICAL across cores (SPMD needs one
    program). Returns (deg[NPAD] float32, Ks[NST], idx[NC,128,NB] int32,
    dstloc[NC,128,NB] float32)."""
    src_all = np.asarray(edge_index[0], dtype=np.int64)
    dst_all = np.asarray(edge_index[1], dtype=np.int64)

    # degree includes the self-loop; the self-loop itself is NOT put in the
    # edge lists - its contribution (dis^2 * h_own) is computed from the
    # residual tile inside the kernel epilogue instead of being gathered.
    deg = (np.bincount(dst_all, minlength=NPAD)
           + np.concatenate([np.ones(N), np.zeros(NPAD - N)])).astype(np.float32)

    core = dst_all // NV
    stl = (dst_all % NV) // 256          # supertile id within core, [0, NST)

    counts = np.zeros((NCORES, NST), np.int64)
    np.add.at(counts, (core, stl), 1)
    # same number of blocks per supertile on every core
    ks = np.ceil(counts.max(axis=0) / 128).astype(np.int64)
    ks = np.maximum(ks, 1)
    nb = int(ks.sum())
    bs = np.zeros(NST, np.int64)
    bs[1:] = np.cumsum(ks)[:-1]

    idx = np.zeros((NCORES, 128, nb), np.int32)
    dstloc = np.full((NCORES, 128, nb), -1.0, np.float32)
    for c in range(NCORES):
        m = core == c
        s_c = src_all[m]
        d_c = dst_all[m]
        stl_c = stl[m]
        # sort by (supertile, src) - src order improves HBM locality
        order = np.lexsort((s_c, stl_c))
        s_c, d_c, stl_c = s_c[order], d_c[order], stl_c[order]
        seg_starts = np.searchsorted(stl_c, np.arange(NST))
        j = np.arange(len(s_c)) - seg_starts[stl_c]
        kb = bs[stl_c] + j // 128
        lane = j % 128
        idx[c, lane, kb] = s_c
        dstloc[c, lane, kb] = (d_c % NV) - stl_c * 256.0
    return deg, ks, idx, dstloc


# ------------------------------------------------------------ kernel builders

def _build_prep():
    """Per-core: dis = where(deg>0, rsqrt(max(deg,1)), 0); htilde0 = dis*x."""
    nc = bacc.Bacc("TRN2")
    deg_in = nc.dram_tensor("deg", [128, NT], F32, kind="ExternalInput")
    x_in = nc.dram_tensor("x", [NV, D], F32, kind="ExternalInput")
    dis_out = nc.dram_tensor("dis", [128, NT], F32, kind="ExternalOutput")
    ht_out = nc.dram_tensor("ht", [NV, D], F32, kind="ExternalOutput")
    with tile.TileContext(nc) as tc, ExitStack() as ctx:
        pool = ctx.enter_context(tc.tile_pool(name="sb", bufs=1))
        ep = ctx.enter_context(tc.tile_pool(name="ep", bufs=4))
        deg_sb = pool.tile([128, NT], F32)
        nc.sync.dma_start(deg_sb[:], deg_in[:])
        mask = pool.tile([128, NT], F32)
        nc.vector.tensor_scalar(out=mask[:], in0=deg_sb[:], scalar1=0.0,
                                scalar2=None, op0=mybir.AluOpType.is_gt)
        mx = pool.tile([128, NT], F32)
        nc.vector.tensor_scalar(out=mx[:], in0=deg_sb[:], scalar1=1.0,
                                scalar2=None, op0=mybir.AluOpType.max)
        sq = pool.tile([128, NT], F32)
        nc.scalar.activation(sq[:], mx[:], mybir.ActivationFunctionType.Sqrt)
        rc = pool.tile([128, NT], F32)
        nc.vector.reciprocal(rc[:], sq[:])
        dis = pool.tile([128, NT], F32)
        nc.vector.tensor_tensor(out=dis[:], in0=rc[:], in1=mask[:],
                                op=mybir.AluOpType.mult)
        nc.sync.dma_start(dis_out[:], dis[:])
        for t in range(NT):
            xt = ep.tile([128, D], F32)
            nc.sync.dma_start(xt[:], x_in[t * 128:(t + 1) * 128, :])
            ht = ep.tile([128, D], F32)
            nc.vector.tensor_scalar(out=ht[:], in0=xt[:],
                                    scalar1=dis[:, t:t + 1], scalar2=None,
                                    op0=mybir.AluOpType.mult)
            nc.sync.dma_start(ht_out[t * 128:(t + 1) * 128, :], ht[:])
    nc.finalize()
    return nc


GRP = 8  # gather blocks per SBUF group tile (one slot-reuse wait per group)


def _build_layer(ks, last, msg_bufs=24, oh_bufs=6):
    """One GCN layer. last=False: outputs h (relu(conv)+res) and htilde=dis*h.
    last=True: second layer fused with the classifier head, outputs logits.

    Self-loop contribution is NOT gathered: z = (agg + dis^2*res) @ W.T via a
    second accumulating matmul off the transposed scaled residual."""
    ks = [int(k) for k in ks]
    nb = int(sum(ks))
    ng = (nb + GRP - 1) // GRP
    nc = bacc.Bacc("TRN2")
    table = nc.dram_tensor("table", [NPAD, D], MMDT, kind="ExternalInput")
    idxs = nc.dram_tensor("idxs", [128, nb], I32, kind="ExternalInput")
    dstl = nc.dram_tensor("dstl", [128, nb], F32, kind="ExternalInput")
    res_in = nc.dram_tensor("res", [NV, D], F32, kind="ExternalInput")
    dis_in = nc.dram_tensor("dis", [128, NT], F32, kind="ExternalInput")
    wt_in = nc.dram_tensor("wt", [D, H], F32, kind="ExternalInput")    # W.T
    bb_in = nc.dram_tensor("bb", [128, H], F32, kind="ExternalInput")  # bias bcast
    iota_in = nc.dram_tensor("iota", [128, 256], F32, kind="ExternalInput")
    id_in = nc.dram_tensor("ident", [128, 128], F32, kind="ExternalInput")
    if last:
        wl_in = nc.dram_tensor("wl", [H, C], F32, kind="ExternalInput")   # Wlin.T
        bl_in = nc.dram_tensor("bl", [128, C], F32, kind="ExternalInput")
        out_lg = nc.dram_tensor("outlg", [NV, C], F32, kind="ExternalOutput")
    else:
        out_h = nc.dram_tensor("outh", [NV, D], F32, kind="ExternalOutput")
        out_ht = nc.dram_tensor("outht", [NV, D], F32, kind="ExternalOutput")

    # block -> supertile map and group tiling
    st_of = []
    for s in range(NST):
        st_of += [s] * ks[s]
    bstart = {}
    pos = 0
    for s in range(NST):
        bstart[s] = pos
        pos += ks[s]

    with tile.TileContext(nc) as tc, ExitStack() as ctx:
        const = ctx.enter_context(tc.tile_pool(name="const", bufs=1))
        msgp = ctx.enter_context(tc.tile_pool(name="msg", bufs=msg_bufs))
        ohp = ctx.enter_context(tc.tile_pool(name="oh", bufs=oh_bufs))
        aggp = ctx.enter_context(tc.tile_pool(name="agg", bufs=3))
        ep = ctx.enter_context(tc.tile_pool(name="ep", bufs=6))
        psum_st = ctx.enter_context(tc.tile_pool(name="pst", bufs=2, space="PSUM"))
        psum_z = ctx.enter_context(tc.tile_pool(name="pz", bufs=2, space="PSUM"))
        psum_t = ctx.enter_context(tc.tile_pool(name="ptr", bufs=2, space="PSUM"))
        if last:
            psum_l = ctx.enter_context(tc.tile_pool(name="plg", bufs=2, space="PSUM"))

        idx_sb = const.tile([128, nb], I32)
        nc.sync.dma_start(idx_sb[:], idxs[:])
        dstl_sb = const.tile([128, nb], F32)
        nc.sync.dma_start(dstl_sb[:], dstl[:])
        iota_sb = const.tile([128, 256], F32)
        nc.sync.dma_start(iota_sb[:], iota_in[:])
        dis_sb = const.tile([128, NT], F32)
        nc.sync.dma_start(dis_sb[:], dis_in[:])
        wt_sb = const.tile([D, H], F32)
        nc.sync.dma_start(wt_sb[:], wt_in[:])
        bb_sb = const.tile([128, H], F32)
        nc.sync.dma_start(bb_sb[:], bb_in[:])
        id_sb = const.tile([128, 128], F32)
        nc.sync.dma_start(id_sb[:], id_in[:])
        if last:
            wl_sb = const.tile([H, C], F32)
            nc.sync.dma_start(wl_sb[:], wl_in[:])
            bl_sb = const.tile([128, C], F32)
            nc.sync.dma_start(bl_sb[:], bl_in[:])

        def epilogue(s, agg):
            for t2 in range(2):
                t = 2 * s + t2
                res_t = ep.tile([128, D], F32)
                nc.sync.dma_start(res_t[:], res_in[t * 128:(t + 1) * 128, :])
                sres = ep.tile([128, D], F32)
                nc.vector.tensor_scalar(out=sres[:], in0=res_t[:],
                                        scalar1=dis_sb[:, t:t + 1], scalar2=None,
                                        op0=mybir.AluOpType.mult)
                tp = psum_t.tile([128, D], F32)
                nc.tensor.transpose(tp[:], sres[:], id_sb[:])
                sresT = ep.tile([128, D], F32)
                nc.vector.tensor_copy(sresT[:], tp[:])
                z = psum_z.tile([128, H], F32)
                nc.tensor.matmul(z[:], lhsT=agg[:, t2 * 128:(t2 + 1) * 128],
                                 rhs=wt_sb[:], start=True, stop=False)
                nc.tensor.matmul(z[:], lhsT=sresT[:], rhs=wt_sb[:],
                                 start=False, stop=True)
                zs = ep.tile([128, H], F32)
                nc.vector.tensor_scalar(out=zs[:], in0=z[:],
                                        scalar1=dis_sb[:, t:t + 1], scalar2=None,
                                        op0=mybir.AluOpType.mult)
                zb = ep.tile([128, H], F32)
                nc.vector.tensor_tensor(out=zb[:], in0=zs[:], in1=bb_sb[:],
                                        op=mybir.AluOpType.add)
                hr = ep.tile([128, H], F32)
                nc.scalar.activation(hr[:], zb[:], mybir.ActivationFunctionType.Relu)
                h = ep.tile([128, H], F32)
                nc.vector.tensor_tensor(out=h[:], in0=hr[:], in1=res_t[:],
                                        op=mybir.AluOpType.add)
                if not last:
                    nc.sync.dma_start(out_h[t * 128:(t + 1) * 128, :], h[:])
                    ht = ep.tile([128, H], F32)
                    nc.vector.tensor_scalar(out=ht[:], in0=h[:],
                                            scalar1=dis_sb[:, t:t + 1],
                                            scalar2=None,
                                            op0=mybir.AluOpType.mult)
                    nc.sync.dma_start(out_ht[t * 128:(t + 1) * 128, :], ht[:])
                else:
                    tp2 = psum_t.tile([128, H], F32, tag="tp")
                    nc.tensor.transpose(tp2[:], h[:], id_sb[:])
                    hT = ep.tile([128, H], F32)
                    nc.vector.tensor_copy(hT[:], tp2[:])
                    lg = psum_l.tile([128, C], F32)
                    nc.tensor.matmul(lg[:], lhsT=hT[:], rhs=wl_sb[:],
                                     start=True, stop=True)
                    lo = ep.tile([128, C], F32)
                    nc.vector.tensor_tensor(out=lo[:], in0=lg[:], in1=bl_sb[:],
                                            op=mybir.AluOpType.add)
                    nc.sync.dma_start(out_lg[t * 128:(t + 1) * 128, :], lo[:])

        ps = None
        for g in range(ng):
            lo_kb = g * GRP
            hi_kb = min(nb, lo_kb + GRP)
            cnt = hi_kb - lo_kb
            msgs = []
            for j in range(cnt):
                kb = lo_kb + j
                msg_t = msgp.tile([128, D], MMDT)
                nc.gpsimd.indirect_dma_start(
                    out=msg_t[:], out_offset=None,
                    in_=table[:, :],
                    in_offset=bass.IndirectOffsetOnAxis(
                        ap=idx_sb[:, kb:kb + 1], axis=0))
                msgs.append(msg_t)
            oh_g = ohp.tile([128, GRP * 256], MMDT)
            dsl = dstl_sb[:, lo_kb:hi_kb].to_broadcast([128, cnt, 256])
            io_ap = iota_sb[:]
            io_b = bass.AP(io_ap.tensor, io_ap.offset,
                           [io_ap.ap[0], [0, cnt], io_ap.ap[1]])
            oh_view = oh_g[:, :cnt * 256]
            oh3 = bass.AP(oh_view.tensor, oh_view.offset,
                          [oh_view.ap[0], [256, cnt], [1, 256]])
            nc.vector.tensor_tensor(out=oh3, in0=dsl, in1=io_b,
                                    op=mybir.AluOpType.is_equal)
            for j in range(cnt):
                kb = lo_kb + j
                s = st_of[kb]
                jj = kb - bstart[s]
                if jj == 0:
                    ps = psum_st.tile([128, 256], F32)
                nc.tensor.matmul(ps[:], lhsT=msgs[j][:],
                                 rhs=oh_g[:, j * 256:(j + 1) * 256],
                                 start=(jj == 0), stop=(jj == ks[s] - 1))
                if jj == ks[s] - 1:
                    agg = aggp.tile([128, 256], F32)
                    nc.vector.tensor_copy(agg[:], ps[:])
                    epilogue(s, agg)
    nc.finalize()
    return nc


# ------------------------------------------------------------------- driver

def _pad_rows(a, rows):
    out = np.zeros((rows, a.shape[1]), dtype=a.dtype)
    out[: a.shape[0]] = a
    return out


_cache = {}


def kernel(x, edge_index, W1, b1, W2, b2, Wlin, blin):
    x = np.asarray(x, dtype=np.float32)
    W1 = np.asarray(W1, dtype=np.float32)
    b1 = np.asarray(b1, dtype=np.float32)
    W2 = np.asarray(W2, dtype=np.float32)
    b2 = np.asarray(b2, dtype=np.float32)
    Wlin = np.asarray(Wlin, dtype=np.float32)
    blin = np.asarray(blin, dtype=np.float32)

    deg, ks, idx, dstloc = _prep_graph(edge_index)
    xp = _pad_rows(x, NPAD)
    iota = np.tile(np.arange(256, dtype=np.float32), (128, 1))
    ident = np.eye(128, dtype=np.float32)
    cores = list(range(NCORES))

    key = tuple(int(k) for k in ks)
    if _cache.get("key") != key:
        _cache.clear()
        _cache["key"] = key
        _cache["prep"] = _build_prep()
        _cache["l1"] = _build_layer(ks, last=False)
        _cache["l2"] = _build_layer(ks, last=True)

    # deg laid out [128, NT] with deg_sb[p, t] = deg[tile t, lane p]
    deg_pc = deg.reshape(NCORES, NT, 128).transpose(0, 2, 1).copy()
    x_pc = xp.reshape(NCORES, NV, D)

    # ---- launch 0: prep
    in0 = [{"deg": deg_pc[c], "x": x_pc[c]} for c in cores]
    r0 = run_bass_kernel_spmd(_cache["prep"], in0, cores)
    dis_pc = np.stack([r0.results[c]["dis"] for c in cores])      # [NC,128,NT]
    ht0 = np.concatenate([r0.results[c]["ht"] for c in cores])    # [NPAD, D]

    # ---- launch 1: layer 1
    in1 = [{"table": ht0, "idxs": idx[c], "dstl": dstloc[c],
            "res": x_pc[c], "dis": dis_pc[c], "wt": W1.T.copy(),
            "bb": np.tile(b1, (128, 1)), "iota": iota, "ident": ident}
           for c in cores]
    r1 = run_bass_kernel_spmd(_cache["l1"], in1, cores)
    h1_pc = np.stack([r1.results[c]["outh"] for c in cores])
    ht1 = np.concatenate([r1.results[c]["outht"] for c in cores])

    # ---- launch 2: layer 2 + head
    in2 = [{"table": ht1, "idxs": idx[c], "dstl": dstloc[c],
            "res": h1_pc[c], "dis": dis_pc[c], "wt": W2.T.copy(),
            "bb": np.tile(b2, (128, 1)), "iota": iota,
            "wl": Wlin.T.copy(), "bl": np.tile(blin, (128, 1)),
            "ident": ident} for c in cores]
    r2 = run_bass_kernel_spmd(_cache["l2"], in2, cores)
    logits = np.concatenate([r2.results[c]["outlg"] for c in cores])
    return logits[:N].astype(np.float32)



# revision 3
# speedup vs baseline: 1.2011x; 1.2011x over previous
"""DiffusionGCN (2-layer GCN + linear head) on 8 Trainium2 NeuronCores.

Strategy (graph/data parallel):
  - Nodes sharded across 8 cores (12544 padded nodes each); edges partitioned
    by destination core, grouped by destination supertile (256 nodes).
  - Symmetric-norm trick: out[v] = dis[v] * sum_{e: dst=v} (dis[src] * h[src]),
    so the source-side scale is folded into the gather table (htilde = dis*h)
    and the dest-side scale is applied after aggregation. W commutes with the
    aggregation and is applied after the segment-sum on the core's own shard.
  - Gathered source features are fetched with bulk `dma_gather` (SWDGE): one
    instruction gathers up to CG*128 rows, amortizing the ~1us fixed SWDGE
    cost that dominates per-row indirect DMA. dma_gather indexes are int16,
    so the node table is split into 4 ranges of 25088 rows; each edge is
    assigned to the gather stream of its source range. Streams are chunked;
    blocks are consumed supertile-major so each supertile's PSUM accumulation
    (one-hot scatter matmuls) closes as soon as its 4 range segments arrive.
  - Segment-sum via one-hot matmuls: per 128-edge block, onehot[e, j] =
    (dstlocal[e] == j) over a 256-wide supertile (single DVE is_equal over a
    group of GRP blocks); PE accumulates msg^T @ onehot into PSUM.
  - 2 SPMD launches: layer 1, layer 2 + classifier head. Host computes deg ->
    dis and htilde0 = dis*x (cheap numpy), and re-shards between launches.
"""

import os
import sys
from contextlib import ExitStack

import numpy as np

for _p in ("/opt/trn_rl_repo", "/root/.axon_site/_ro/trn_rl_repo"):
    if os.path.isdir(_p) and _p not in sys.path:
        sys.path.insert(0, _p)

import concourse.bacc as bacc
import concourse.bass as bass
import concourse.mybir as mybir
import concourse.tile as tile
from concourse.bass_utils import run_bass_kernel_spmd

F32 = mybir.dt.float32
F32R = mybir.dt.float32r
I16 = mybir.dt.int16

N = 100000
E = 1600000
D = 128
H = 128
C = 64
NCORES = 8
NPAD = 100352            # 8 * 12544
NV = NPAD // NCORES      # 12544 nodes per core
NT = NV // 128           # 98 tiles per core
NST = NT // 2            # 49 supertiles (256 nodes) per core
NRANGE = 4               # int16 gather-index ranges
RV = NPAD // NRANGE      # 25088 rows per range table
CG = 32                  # blocks (of 128 edges) per dma_gather chunk
GRP = 8                  # blocks per one-hot DVE build

MMDT = F32R              # dtype of scatter-matmul operands


# ----------------------------------------------------------------- block plan

def _block_plan(ks):
    """Derive per-block/per-chunk metadata from ks[NST, NRANGE] (blocks per
    (supertile, range) segment; identical across cores).

    Consumption (emission) order: supertile-major, range inner, block last.
    Stream order (per range r): supertile-major concatenation of segments.
    Returns dict with:
      nb, blk_r[nb], blk_s[nb], stream_pos[nb], first[nb], last[nb],
      stream_off[NRANGE+1], chunks=[(r, b0, b1, col_off)], chunk_of[nb],
      gb_of_stream[nb]
    """
    ks = np.asarray(ks, np.int64).reshape(NST, NRANGE)
    flat_ks = ks.reshape(-1)
    nb = int(flat_ks.sum())
    blk_s = np.repeat(np.arange(NST), ks.sum(axis=1))
    blk_r = np.repeat(np.tile(np.arange(NRANGE), NST), flat_ks)
    stream_pos = np.zeros(nb, np.int64)
    for r in range(NRANGE):
        sel = blk_r == r
        stream_pos[sel] = np.arange(int(sel.sum()))
    tot_r = [int((blk_r == r).sum()) for r in range(NRANGE)]
    stream_off = np.zeros(NRANGE + 1, np.int64)
    stream_off[1:] = np.cumsum(tot_r)
    gb_of_stream = np.zeros(nb, np.int64)
    gb_of_stream[stream_off[blk_r] + stream_pos] = np.arange(nb)
    # first/last block of each supertile in consumption order
    per_s = ks.sum(axis=1)
    s_start = np.zeros(NST, np.int64)
    s_start[1:] = np.cumsum(per_s)[:-1]
    first = np.zeros(nb, bool)
    last = np.zeros(nb, bool)
    first[s_start] = True
    last[s_start + per_s - 1] = True
    # chunks per range, with idx-const column offsets (8 cols per block)
    chunks = []
    col = 0
    chunk_of = np.zeros(nb, np.int64)
    for r in range(NRANGE):
        nchk = (tot_r[r] + CG - 1) // CG
        for k in range(nchk):
            b0, b1 = k * CG, min((k + 1) * CG, tot_r[r])
            sel = (blk_r == r) & (stream_pos >= b0) & (stream_pos < b1)
            chunk_of[sel] = len(chunks)
            chunks.append((r, b0, b1, col))
            col += (b1 - b0) * 8
    assert col == nb * 8
    return dict(nb=nb, ks=ks, blk_r=blk_r, blk_s=blk_s,
                stream_pos=stream_pos, first=first, last=last,
                stream_off=stream_off, chunks=chunks, chunk_of=chunk_of,
                gb_of_stream=gb_of_stream)


# ----------------------------------------------------------------- host prep

def _prep_graph(edge_index):
    """Partition/sort edges into the (supertile, range) block grid. Returns
    (deg[NPAD] f32, plan, idx[NC,128,nb*8] i16, dstloc[NC,128,nb] f32)."""
    src_all = np.asarray(edge_index[0], dtype=np.int64)
    dst_all = np.asarray(edge_index[1], dtype=np.int64)

    # degree includes the self-loop; the self-loop itself is NOT in the edge
    # lists - its contribution (dis^2 * h_own) comes from the residual tile in
    # the kernel epilogue.
    deg = (np.bincount(dst_all, minlength=NPAD)
           + np.concatenate([np.ones(N), np.zeros(NPAD - N)])).astype(np.float32)

    core = dst_all // NV
    stl = (dst_all % NV) // 256
    rr = src_all // RV

    counts = np.zeros((NCORES, NST, NRANGE), np.int64)
    np.add.at(counts, (core, stl, rr), 1)
    ks = np.ceil(counts.max(axis=0) / 128).astype(np.int64)
    ks = np.maximum(ks, 1)
    plan = _block_plan(ks)
    nb = plan["nb"]
    flat_ks = ks.reshape(-1)
    gb_base = np.zeros(NST * NRANGE, np.int64)
    gb_base[1:] = np.cumsum(flat_ks)[:-1]
    gb_base = gb_base.reshape(NST, NRANGE)

    lane_src = np.full((NCORES, nb, 128), -1, np.int64)
    lane_dst = np.full((NCORES, nb, 128), -1.0, np.float32)
    for c in range(NCORES):
        m = core == c
        s_c = src_all[m]
        d_c = dst_all[m]
        st_c = stl[m]
        r_c = rr[m]
        # sort by (supertile, range, src) - src order improves HBM locality
        order = np.lexsort((s_c, r_c, st_c))
        s_c, d_c, st_c, r_c = s_c[order], d_c[order], st_c[order], r_c[order]
        loc = s_c - r_c * RV
        key = st_c * NRANGE + r_c
        seg_starts = np.searchsorted(key, np.arange(NST * NRANGE))
        j = np.arange(len(s_c)) - seg_starts[key]
        gb = gb_base[st_c, r_c] + j // 128
        lane = j % 128
        lane_src[c, gb, lane] = loc
        lane_dst[c, gb, lane] = (d_c % NV) - st_c * 256.0
        # pad unfilled lanes with the segment's last real index (keeps HBM
        # row locality); dstloc stays -1 so the one-hot kills them.
        segc = counts[c].reshape(-1)
        pad = np.zeros(NST * NRANGE, np.int64)
        nz = segc > 0
        pad[nz] = loc[(seg_starts + segc - 1)[nz]]
        blk_pad = np.repeat(pad, flat_ks)
        msk = lane_src[c] < 0
        lane_src[c][msk] = np.broadcast_to(blk_pad[:, None], (nb, 128))[msk]

    # idx const [NC, 128, nb*8]: per-chunk 16-lane wrap, replicated x8
    lane_src_stream = lane_src[:, plan["gb_of_stream"], :]
    idx = np.zeros((NCORES, 128, nb * 8), np.int16)
    so = plan["stream_off"]
    for (r, b0, b1, col) in plan["chunks"]:
        cb = b1 - b0
        sl = lane_src_stream[:, so[r] + b0:so[r] + b1, :]      # [NC, cb, 128]
        flat = sl.reshape(NCORES, cb * 128)
        wrapped = flat.reshape(NCORES, cb * 8, 16).transpose(0, 2, 1)
        idx[:, :, col:col + cb * 8] = np.tile(wrapped, (1, 8, 1)).astype(np.int16)

    dstloc = lane_dst.transpose(0, 2, 1).copy()                # [NC, 128, nb]
    return deg, plan, idx, dstloc


# ------------------------------------------------------------ kernel builder

def _build_layer(ks_key, last, msg_bufs=6, oh_bufs=4):
    """One GCN layer. last=False: outputs h (relu(conv)+res) and htilde=dis*h.
    last=True: second layer fused with the classifier head, outputs logits.

    Self-loop contribution is not gathered: z = (agg + dis^2*res) @ W.T via a
    second accumulating matmul off the transposed scaled residual."""
    plan = _block_plan(np.asarray(ks_key, np.int64).reshape(NST, NRANGE))
    nb = plan["nb"]
    nc = bacc.Bacc("TRN2")
    tables = [nc.dram_tensor(f"table{r}", [RV, D], MMDT, kind="ExternalInput")
              for r in range(NRANGE)]
    idxs = nc.dram_tensor("idxs", [128, nb * 8], I16, kind="ExternalInput")
    dstl = nc.dram_tensor("dstl", [128, nb], F32, kind="ExternalInput")
    res_in = nc.dram_tensor("res", [NV, D], F32, kind="ExternalInput")
    dis_in = nc.dram_tensor("dis", [128, NT], F32, kind="ExternalInput")
    wt_in = nc.dram_tensor("wt", [D, H], F32, kind="ExternalInput")    # W.T
    bb_in = nc.dram_tensor("bb", [128, H], F32, kind="ExternalInput")  # bias bcast
    iota_in = nc.dram_tensor("iota", [128, 256], F32, kind="ExternalInput")
    id_in = nc.dram_tensor("ident", [128, 128], F32, kind="ExternalInput")
    if last:
        wl_in = nc.dram_tensor("wl", [H, C], F32, kind="ExternalInput")   # Wlin.T
        bl_in = nc.dram_tensor("bl", [128, C], F32, kind="ExternalInput")
        out_lg = nc.dram_tensor("outlg", [NV, C], F32, kind="ExternalOutput")
    else:
        out_h = nc.dram_tensor("outh", [NV, D], F32, kind="ExternalOutput")
        out_ht = nc.dram_tensor("outht", [NV, D], F32, kind="ExternalOutput")

    with tile.TileContext(nc) as tc, ExitStack() as ctx:
        const = ctx.enter_context(tc.tile_pool(name="const", bufs=1))
        msgp = ctx.enter_context(tc.tile_pool(name="msg", bufs=msg_bufs))
        ohp = ctx.enter_context(tc.tile_pool(name="oh", bufs=oh_bufs))
        aggp = ctx.enter_context(tc.tile_pool(name="agg", bufs=3))
        ep = ctx.enter_context(tc.tile_pool(name="ep", bufs=6))
        psum_st = ctx.enter_context(tc.tile_pool(name="pst", bufs=2, space="PSUM"))
        psum_z = ctx.enter_context(tc.tile_pool(name="pz", bufs=2, space="PSUM"))
        psum_t = ctx.enter_context(tc.tile_pool(name="ptr", bufs=2, space="PSUM"))
        if last:
            psum_l = ctx.enter_context(tc.tile_pool(name="plg", bufs=2, space="PSUM"))

        idx_sb = const.tile([128, nb * 8], I16)
        nc.sync.dma_start(idx_sb[:], idxs[:])
        dstl_sb = const.tile([128, nb], F32)
        nc.sync.dma_start(dstl_sb[:], dstl[:])
        iota_sb = const.tile([128, 256], F32)
        nc.sync.dma_start(iota_sb[:], iota_in[:])
        dis_sb = const.tile([128, NT], F32)
        nc.sync.dma_start(dis_sb[:], dis_in[:])
        wt_sb = const.tile([D, H], F32)
        nc.sync.dma_start(wt_sb[:], wt_in[:])
        bb_sb = const.tile([128, H], F32)
        nc.sync.dma_start(bb_sb[:], bb_in[:])
        id_sb = const.tile([128, 128], F32)
        nc.sync.dma_start(id_sb[:], id_in[:])
        if last:
            wl_sb = const.tile([H, C], F32)
            nc.sync.dma_start(wl_sb[:], wl_in[:])
            bl_sb = const.tile([128, C], F32)
            nc.sync.dma_start(bl_sb[:], bl_in[:])

        def epilogue(s, agg):
            for t2 in range(2):
                t = 2 * s + t2
                res_t = ep.tile([128, D], F32)
                nc.sync.dma_start(res_t[:], res_in[t * 128:(t + 1) * 128, :])
                sres = ep.tile([128, D], F32)
                nc.vector.tensor_scalar(out=sres[:], in0=res_t[:],
                                        scalar1=dis_sb[:, t:t + 1], scalar2=None,
                                        op0=mybir.AluOpType.mult)
                tp = psum_t.tile([128, D], F32)
                nc.tensor.transpose(tp[:], sres[:], id_sb[:])
                sresT = ep.tile([128, D], F32)
                nc.vector.tensor_copy(sresT[:], tp[:])
                z = psum_z.tile([128, H], F32)
                nc.tensor.matmul(z[:], lhsT=agg[:, t2 * 128:(t2 + 1) * 128],
                                 rhs=wt_sb[:], start=True, stop=False)
                nc.tensor.matmul(z[:], lhsT=sresT[:], rhs=wt_sb[:],
                                 start=False, stop=True)
                zs = ep.tile([128, H], F32)
                nc.vector.tensor_scalar(out=zs[:], in0=z[:],
                                        scalar1=dis_sb[:, t:t + 1], scalar2=None,
                                        op0=mybir.AluOpType.mult)
                zb = ep.tile([128, H], F32)
                nc.vector.tensor_tensor(out=zb[:], in0=zs[:], in1=bb_sb[:],
                                        op=mybir.AluOpType.add)
                hr = ep.tile([128, H], F32)
                nc.scalar.activation(hr[:], zb[:], mybir.ActivationFunctionType.Relu)
                h = ep.tile([128, H], F32)
                nc.vector.tensor_tensor(out=h[:], in0=hr[:], in1=res_t[:],
                                        op=mybir.AluOpType.add)
                if not last:
                    nc.sync.dma_start(out_h[t * 128:(t + 1) * 128, :], h[:])
                    ht = ep.tile([128, H], F32)
                    nc.vector.tensor_scalar(out=ht[:], in0=h[:],
                                            scalar1=dis_sb[:, t:t + 1],
                                            scalar2=None,
                                            op0=mybir.AluOpType.mult)
                    nc.sync.dma_start(out_ht[t * 128:(t + 1) * 128, :], ht[:])
                else:
                    tp2 = psum_t.tile([128, H], F32, tag="tp")
                    nc.tensor.transpose(tp2[:], h[:], id_sb[:])
                    hT = ep.tile([128, H], F32)
                    nc.vector.tensor_copy(hT[:], tp2[:])
                    lg = psum_l.tile([128, C], F32)
                    nc.tensor.matmul(lg[:], lhsT=hT[:], rhs=wl_sb[:],
                                     start=True, stop=True)
                    lo = ep.tile([128, C], F32)
                    nc.vector.tensor_tensor(out=lo[:], in0=lg[:], in1=bl_sb[:],
                                            op=mybir.AluOpType.add)
                    nc.sync.dma_start(out_lg[t * 128:(t + 1) * 128, :], lo[:])

        chunks = plan["chunks"]
        blk_r = plan["blk_r"]
        chunk_of = plan["chunk_of"]
        stream_pos = plan["stream_pos"]
        first = plan["first"]
        last_b = plan["last"]
        blk_s = plan["blk_s"]

        cur = {r: (-1, None) for r in range(NRANGE)}  # r -> (chunk id, tile)
        ps = None
        ng = (nb + GRP - 1) // GRP
        for g in range(ng):
            lo_gb = g * GRP
            hi_gb = min(nb, lo_gb + GRP)
            cnt = hi_gb - lo_gb
            # resolve (and issue) gathers for this group's blocks
            resolved = []
            for gb in range(lo_gb, hi_gb):
                r = int(blk_r[gb])
                ck = int(chunk_of[gb])
                if cur[r][0] != ck:
                    (rr_, b0, b1, col) = chunks[ck]
                    assert rr_ == r
                    nblk = b1 - b0
                    t = msgp.tile([128, CG * D], MMDT)
                    m = t[:]
                    out3 = bass.AP(m.tensor, m.offset,
                                   [m.ap[0], [D, nblk], [1, D]])
                    lanes = nblk * 128
                    nc.gpsimd.dma_gather(
                        out3, tables[r][:, :],
                        idx_sb[:, col:col + nblk * 8],
                        lanes, lanes, D, single_packet=False)
                    cur[r] = (ck, t)
                t = cur[r][1]
                resolved.append((gb, t, int(stream_pos[gb]) % CG))
            # one-hot for the whole group in a single DVE op
            oh_g = ohp.tile([128, GRP * 256], MMDT)
            dsl = dstl_sb[:, lo_gb:hi_gb].to_broadcast([128, cnt, 256])
            io_ap = iota_sb[:]
            io_b = bass.AP(io_ap.tensor, io_ap.offset,
                           [io_ap.ap[0], [0, cnt], io_ap.ap[1]])
            oh_view = oh_g[:, :cnt * 256]
            oh3 = bass.AP(oh_view.tensor, oh_view.offset,
                          [oh_view.ap[0], [256, cnt], [1, 256]])
            nc.vector.tensor_tensor(out=oh3, in0=dsl, in1=io_b,
                                    op=mybir.AluOpType.is_equal)
            # scatter matmuls
            for j, (gb, t, colk) in enumerate(resolved):
                if first[gb]:
                    ps = psum_st.tile([128, 256], F32)
                nc.tensor.matmul(ps[:], lhsT=t[:, colk * D:(colk + 1) * D],
                                 rhs=oh_g[:, j * 256:(j + 1) * 256],
                                 start=bool(first[gb]), stop=bool(last_b[gb]))
                if last_b[gb]:
                    s = int(blk_s[gb])
                    agg = aggp.tile([128, 256], F32)
                    nc.vector.tensor_copy(agg[:], ps[:])
                    epilogue(s, agg)
    nc.finalize()
    return nc


# ------------------------------------------------------------------- driver

def _pad_rows(a, rows):
    out = np.zeros((rows, a.shape[1]), dtype=a.dtype)
    out[: a.shape[0]] = a
    return out


_cache = {}


def _host_prep(x, edge_index):
    deg, plan, idx, dstloc = _prep_graph(edge_index)
    dis = np.where(deg > 0, 1.0 / np.sqrt(np.maximum(deg, 1.0)),
                   0.0).astype(np.float32)
    xp = _pad_rows(np.asarray(x, np.float32), NPAD)
    ht0 = (dis[:, None] * xp).astype(np.float32)
    # dis laid out [128, NT] per core with dis_pc[c, p, t] = dis[core c, tile t, lane p]
    dis_pc = dis.reshape(NCORES, NT, 128).transpose(0, 2, 1).copy()
    return plan, idx, dstloc, dis_pc, xp, ht0


def kernel(x, edge_index, W1, b1, W2, b2, Wlin, blin):
    x = np.asarray(x, dtype=np.float32)
    W1 = np.asarray(W1, dtype=np.float32)
    b1 = np.asarray(b1, dtype=np.float32)
    W2 = np.asarray(W2, dtype=np.float32)
    b2 = np.asarray(b2, dtype=np.float32)
    Wlin = np.asarray(Wlin, dtype=np.float32)
    blin = np.asarray(blin, dtype=np.float32)

    plan, idx, dstloc, dis_pc, xp, ht0 = _host_prep(x, edge_index)
    iota = np.tile(np.arange(256, dtype=np.float32), (128, 1))
    ident = np.eye(128, dtype=np.float32)
    cores = list(range(NCORES))
    x_pc = xp.reshape(NCORES, NV, D)

    key = tuple(int(k) for k in plan["ks"].reshape(-1))
    if _cache.get("key") != key:
        _cache.clear()
        _cache["key"] = key
        _cache["l1"] = _build_layer(key, last=False)
        _cache["l2"] = _build_layer(key, last=True)

    def tabs(ht):
        return {f"table{r}": ht[r * RV:(r + 1) * RV] for r in range(NRANGE)}

    # ---- launch 1: layer 1
    in1 = [{**tabs(ht0), "idxs": idx[c], "dstl": dstloc[c],
            "res": x_pc[c], "dis": dis_pc[c], "wt": W1.T.copy(),
            "bb": np.tile(b1, (128, 1)), "iota": iota, "ident": ident}
           for c in cores]
    r1 = run_bass_kernel_spmd(_cache["l1"], in1, cores)
    h1_pc = np.stack([r1.results[c]["outh"] for c in cores])
    ht1 = np.concatenate([r1.results[c]["outht"] for c in cores])

    # ---- launch 2: layer 2 + head
    in2 = [{**tabs(ht1), "idxs": idx[c], "dstl": dstloc[c],
            "res": h1_pc[c], "dis": dis_pc[c], "wt": W2.T.copy(),
            "bb": np.tile(b2, (128, 1)), "iota": iota,
            "wl": Wlin.T.copy(), "bl": np.tile(blin, (128, 1)),
            "ident": ident} for c in cores]
    r2 = run_bass_kernel_spmd(_cache["l2"], in2, cores)
    logits = np.concatenate([r2.results[c]["outlg"] for c in cores])
    return logits[:N].astype(np.float32)


# revision 5
# speedup vs baseline: 1.3939x; 1.1605x over previous
"""DiffusionGCN (2-layer GCN + linear head) on 8 Trainium2 NeuronCores.

Strategy (graph/data parallel):
  - Nodes sharded across 8 cores (12544 padded nodes each); edges partitioned
    by destination core, grouped by destination supertile (256 nodes).
  - Symmetric-norm trick: out[v] = dis[v] * sum_{e: dst=v} (dis[src] * h[src]),
    so the source-side scale is folded into the gather table (htilde = dis*h)
    and the dest-side scale is applied after aggregation. W commutes with the
    aggregation and is applied after the segment-sum on the core's own shard.
  - Gathered source features are fetched with bulk `dma_gather` (SWDGE): one
    instruction gathers up to CG*128 rows, amortizing the ~1us fixed SWDGE
    cost that dominates per-row indirect DMA. dma_gather indexes are int16,
    so the node table is split into 4 ranges of 25088 rows; each edge is
    assigned to the gather stream of its source range. Streams are chunked;
    blocks are consumed supertile-major so each supertile's PSUM accumulation
    (one-hot scatter matmuls) closes as soon as its 4 range segments arrive.
  - Segment-sum via one-hot matmuls: per 128-edge block, onehot[e, j] =
    (dstlocal[e] == j) over a 256-wide supertile (single DVE is_equal over a
    group of GRP blocks); PE accumulates msg^T @ onehot into PSUM.
  - 2 SPMD launches: layer 1, layer 2 + classifier head. Host computes deg ->
    dis and htilde0 = dis*x (cheap numpy), and re-shards between launches.
"""

import os
import sys
from contextlib import ExitStack

import numpy as np

for _p in ("/opt/trn_rl_repo", "/root/.axon_site/_ro/trn_rl_repo"):
    if os.path.isdir(_p) and _p not in sys.path:
        sys.path.insert(0, _p)

import concourse.bacc as bacc
import concourse.bass as bass
import concourse.mybir as mybir
import concourse.tile as tile
from concourse.bass_utils import run_bass_kernel_spmd

F32 = mybir.dt.float32
F32R = mybir.dt.float32r
I16 = mybir.dt.int16

N = 100000
E = 1600000
D = 128
H = 128
C = 64
NCORES = 8
NPAD = 100352            # 8 * 12544
NV = NPAD // NCORES      # 12544 nodes per core
NT = NV // 128           # 98 tiles per core
NST = NT // 2            # 49 supertiles (256 nodes) per core
NRANGE = 4               # int16 gather-index ranges
RV = NPAD // NRANGE      # 25088 rows per range table
CG = 32                  # blocks (of 128 edges) per dma_gather chunk
GRP = 8                  # blocks per one-hot DVE build

MMDT = F32R              # dtype of scatter-matmul operands


# ----------------------------------------------------------------- block plan

def _block_plan(ks):
    """Derive per-block/per-chunk metadata from ks[NST, NRANGE] (blocks per
    (supertile, range) segment; identical across cores).

    Consumption (emission) order: supertile-major, range inner, block last.
    Stream order (per range r): supertile-major concatenation of segments.
    Returns dict with:
      nb, blk_r[nb], blk_s[nb], stream_pos[nb], first[nb], last[nb],
      stream_off[NRANGE+1], chunks=[(r, b0, b1, col_off)], chunk_of[nb],
      gb_of_stream[nb]
    """
    ks = np.asarray(ks, np.int64).reshape(NST, NRANGE)
    flat_ks = ks.reshape(-1)
    nb = int(flat_ks.sum())
    blk_s = np.repeat(np.arange(NST), ks.sum(axis=1))
    blk_r = np.repeat(np.tile(np.arange(NRANGE), NST), flat_ks)
    stream_pos = np.zeros(nb, np.int64)
    for r in range(NRANGE):
        sel = blk_r == r
        stream_pos[sel] = np.arange(int(sel.sum()))
    tot_r = [int((blk_r == r).sum()) for r in range(NRANGE)]
    stream_off = np.zeros(NRANGE + 1, np.int64)
    stream_off[1:] = np.cumsum(tot_r)
    gb_of_stream = np.zeros(nb, np.int64)
    gb_of_stream[stream_off[blk_r] + stream_pos] = np.arange(nb)
    # first/last block of each supertile in consumption order
    per_s = ks.sum(axis=1)
    s_start = np.zeros(NST, np.int64)
    s_start[1:] = np.cumsum(per_s)[:-1]
    first = np.zeros(nb, bool)
    last = np.zeros(nb, bool)
    first[s_start] = True
    last[s_start + per_s - 1] = True
    # chunks per range, with idx-const column offsets (8 cols per block)
    chunks = []
    col = 0
    chunk_of = np.zeros(nb, np.int64)
    for r in range(NRANGE):
        nchk = (tot_r[r] + CG - 1) // CG
        for k in range(nchk):
            b0, b1 = k * CG, min((k + 1) * CG, tot_r[r])
            sel = (blk_r == r) & (stream_pos >= b0) & (stream_pos < b1)
            chunk_of[sel] = len(chunks)
            chunks.append((r, b0, b1, col))
            col += (b1 - b0) * 8
    assert col == nb * 8
    return dict(nb=nb, ks=ks, blk_r=blk_r, blk_s=blk_s,
                stream_pos=stream_pos, first=first, last=last,
                stream_off=stream_off, chunks=chunks, chunk_of=chunk_of,
                gb_of_stream=gb_of_stream)


# ----------------------------------------------------------------- host prep

def _prep_graph(edge_index):
    """Partition/sort edges into the (supertile, range) block grid. Returns
    (deg[NPAD] f32, plan, idx[NC,128,nb*8] i16, dstloc[NC,128,nb] f32)."""
    src_all = np.asarray(edge_index[0], dtype=np.int64)
    dst_all = np.asarray(edge_index[1], dtype=np.int64)

    # degree includes the self-loop; the self-loop itself is NOT in the edge
    # lists - its contribution (dis^2 * h_own) comes from the residual tile in
    # the kernel epilogue.
    deg = (np.bincount(dst_all, minlength=NPAD)
           + np.concatenate([np.ones(N), np.zeros(NPAD - N)])).astype(np.float32)

    core = dst_all // NV
    stl = (dst_all % NV) // 256
    rr = src_all // RV

    counts = np.zeros((NCORES, NST, NRANGE), np.int64)
    np.add.at(counts, (core, stl, rr), 1)
    ks = np.ceil(counts.max(axis=0) / 128).astype(np.int64)
    ks = np.maximum(ks, 1)
    plan = _block_plan(ks)
    nb = plan["nb"]
    flat_ks = ks.reshape(-1)
    gb_base = np.zeros(NST * NRANGE, np.int64)
    gb_base[1:] = np.cumsum(flat_ks)[:-1]
    gb_base = gb_base.reshape(NST, NRANGE)

    lane_src = np.full((NCORES, nb, 128), -1, np.int64)
    lane_dst = np.full((NCORES, nb, 128), -1.0, np.float32)
    for c in range(NCORES):
        m = core == c
        s_c = src_all[m]
        d_c = dst_all[m]
        st_c = stl[m]
        r_c = rr[m]
        # sort by (supertile, range, src) - src order improves HBM locality
        order = np.lexsort((s_c, r_c, st_c))
        s_c, d_c, st_c, r_c = s_c[order], d_c[order], st_c[order], r_c[order]
        loc = s_c - r_c * RV
        key = st_c * NRANGE + r_c
        seg_starts = np.searchsorted(key, np.arange(NST * NRANGE))
        j = np.arange(len(s_c)) - seg_starts[key]
        gb = gb_base[st_c, r_c] + j // 128
        lane = j % 128
        lane_src[c, gb, lane] = loc
        lane_dst[c, gb, lane] = (d_c % NV) - st_c * 256.0
        # pad unfilled lanes with the segment's last real index (keeps HBM
        # row locality); dstloc stays -1 so the one-hot kills them.
        segc = counts[c].reshape(-1)
        pad = np.zeros(NST * NRANGE, np.int64)
        nz = segc > 0
        pad[nz] = loc[(seg_starts + segc - 1)[nz]]
        blk_pad = np.repeat(pad, flat_ks)
        msk = lane_src[c] < 0
        lane_src[c][msk] = np.broadcast_to(blk_pad[:, None], (nb, 128))[msk]

    # idx const [NC, 128, nb*8]: per-chunk 16-lane wrap, replicated x8
    lane_src_stream = lane_src[:, plan["gb_of_stream"], :]
    idx = np.zeros((NCORES, 128, nb * 8), np.int16)
    so = plan["stream_off"]
    for (r, b0, b1, col) in plan["chunks"]:
        cb = b1 - b0
        sl = lane_src_stream[:, so[r] + b0:so[r] + b1, :]      # [NC, cb, 128]
        flat = sl.reshape(NCORES, cb * 128)
        wrapped = flat.reshape(NCORES, cb * 8, 16).transpose(0, 2, 1)
        idx[:, :, col:col + cb * 8] = np.tile(wrapped, (1, 8, 1)).astype(np.int16)

    dstloc = lane_dst.transpose(0, 2, 1).copy()                # [NC, 128, nb]
    return deg, plan, idx, dstloc


# ------------------------------------------------------------ kernel builder

def _build_layer(ks_key, last, msg_bufs=5, oh_bufs=4):
    """One GCN layer. last=False: outputs h (relu(conv)+res) and htilde=dis*h.
    last=True: second layer fused with the classifier head, outputs logits.

    Self-loop contribution is not gathered: z = (agg + dis^2*res) @ W.T via a
    second accumulating matmul off the transposed scaled residual."""
    plan = _block_plan(np.asarray(ks_key, np.int64).reshape(NST, NRANGE))
    nb = plan["nb"]
    nc = bacc.Bacc("TRN2", num_swdge_queues=4)
    tables = [nc.dram_tensor(f"table{r}", [RV, D], MMDT, kind="ExternalInput")
              for r in range(NRANGE)]
    idxs = nc.dram_tensor("idxs", [128, nb * 8], I16, kind="ExternalInput")
    dstl = nc.dram_tensor("dstl", [128, nb], F32, kind="ExternalInput")
    res_in = nc.dram_tensor("res", [NV, D], F32, kind="ExternalInput")
    dis_in = nc.dram_tensor("dis", [128, NT], F32, kind="ExternalInput")
    wt_in = nc.dram_tensor("wt", [D, H], F32, kind="ExternalInput")    # W.T
    bb_in = nc.dram_tensor("bb", [128, H], F32, kind="ExternalInput")  # bias bcast
    iota_in = nc.dram_tensor("iota", [128, 256], F32, kind="ExternalInput")
    id_in = nc.dram_tensor("ident", [128, 128], F32, kind="ExternalInput")
    if last:
        wl_in = nc.dram_tensor("wl", [H, C], F32, kind="ExternalInput")   # Wlin.T
        bl_in = nc.dram_tensor("bl", [128, C], F32, kind="ExternalInput")
        out_lg = nc.dram_tensor("outlg", [NV, C], F32, kind="ExternalOutput")
    else:
        out_h = nc.dram_tensor("outh", [NV, D], F32, kind="ExternalOutput")
        out_ht = nc.dram_tensor("outht", [NV, D], F32, kind="ExternalOutput")

    with tile.TileContext(nc) as tc, ExitStack() as ctx:
        const = ctx.enter_context(tc.tile_pool(name="const", bufs=1))
        msgp = ctx.enter_context(tc.tile_pool(name="msg", bufs=msg_bufs))
        ohp = ctx.enter_context(tc.tile_pool(name="oh", bufs=oh_bufs))
        aggp = ctx.enter_context(tc.tile_pool(name="agg", bufs=3))
        ep = ctx.enter_context(tc.tile_pool(name="ep", bufs=6))
        psum_st = ctx.enter_context(tc.tile_pool(name="pst", bufs=2, space="PSUM"))
        psum_z = ctx.enter_context(tc.tile_pool(name="pz", bufs=2, space="PSUM"))
        psum_t = ctx.enter_context(tc.tile_pool(name="ptr", bufs=2, space="PSUM"))
        if last:
            psum_l = ctx.enter_context(tc.tile_pool(name="plg", bufs=2, space="PSUM"))

        idx_sb = const.tile([128, nb * 8], I16)
        nc.sync.dma_start(idx_sb[:], idxs[:])
        dstl_sb = const.tile([128, nb], F32)
        nc.sync.dma_start(dstl_sb[:], dstl[:])
        iota_sb = const.tile([128, 256], F32)
        nc.sync.dma_start(iota_sb[:], iota_in[:])
        dis_sb = const.tile([128, NT], F32)
        nc.sync.dma_start(dis_sb[:], dis_in[:])
        wt_sb = const.tile([D, H], F32)
        nc.sync.dma_start(wt_sb[:], wt_in[:])
        bb_sb = const.tile([128, H], F32)
        nc.sync.dma_start(bb_sb[:], bb_in[:])
        id_sb = const.tile([128, 128], F32)
        nc.sync.dma_start(id_sb[:], id_in[:])
        if last:
            wl_sb = const.tile([H, C], F32)
            nc.sync.dma_start(wl_sb[:], wl_in[:])
            bl_sb = const.tile([128, C], F32)
            nc.sync.dma_start(bl_sb[:], bl_in[:])

        def epilogue(s, agg):
            for t2 in range(2):
                t = 2 * s + t2
                res_t = ep.tile([128, D], F32)
                nc.sync.dma_start(res_t[:], res_in[t * 128:(t + 1) * 128, :])
                sres = ep.tile([128, D], F32)
                nc.scalar.activation(sres[:], res_t[:],
                                     mybir.ActivationFunctionType.Identity,
                                     scale=dis_sb[:, t:t + 1])
                tp = psum_t.tile([128, D], F32)
                nc.tensor.transpose(tp[:], sres[:], id_sb[:])
                sresT = ep.tile([128, D], F32)
                nc.vector.tensor_copy(sresT[:], tp[:])
                z = psum_z.tile([128, H], F32)
                nc.tensor.matmul(z[:], lhsT=agg[:, t2 * 128:(t2 + 1) * 128],
                                 rhs=wt_sb[:], start=True, stop=False)
                nc.tensor.matmul(z[:], lhsT=sresT[:], rhs=wt_sb[:],
                                 start=False, stop=True)
                zs = ep.tile([128, H], F32)
                nc.scalar.activation(zs[:], z[:],
                                     mybir.ActivationFunctionType.Identity,
                                     scale=dis_sb[:, t:t + 1])
                zb = ep.tile([128, H], F32)
                nc.vector.tensor_tensor(out=zb[:], in0=zs[:], in1=bb_sb[:],
                                        op=mybir.AluOpType.add)
                hr = ep.tile([128, H], F32)
                nc.scalar.activation(hr[:], zb[:], mybir.ActivationFunctionType.Relu)
                h = ep.tile([128, H], F32)
                nc.vector.tensor_tensor(out=h[:], in0=hr[:], in1=res_t[:],
                                        op=mybir.AluOpType.add)
                if not last:
                    nc.sync.dma_start(out_h[t * 128:(t + 1) * 128, :], h[:])
                    ht = ep.tile([128, H], F32)
                    nc.scalar.activation(ht[:], h[:],
                                         mybir.ActivationFunctionType.Identity,
                                         scale=dis_sb[:, t:t + 1])
                    nc.sync.dma_start(out_ht[t * 128:(t + 1) * 128, :], ht[:])
                else:
                    tp2 = psum_t.tile([128, H], F32, tag="tp")
                    nc.tensor.transpose(tp2[:], h[:], id_sb[:])
                    hT = ep.tile([128, H], F32)
                    nc.vector.tensor_copy(hT[:], tp2[:])
                    lg = psum_l.tile([128, C], F32)
                    nc.tensor.matmul(lg[:], lhsT=hT[:], rhs=wl_sb[:],
                                     start=True, stop=True)
                    lo = ep.tile([128, C], F32)
                    nc.vector.tensor_tensor(out=lo[:], in0=lg[:], in1=bl_sb[:],
                                            op=mybir.AluOpType.add)
                    nc.sync.dma_start(out_lg[t * 128:(t + 1) * 128, :], lo[:])

        chunks = plan["chunks"]
        blk_r = plan["blk_r"]
        chunk_of = plan["chunk_of"]
        stream_pos = plan["stream_pos"]
        first = plan["first"]
        last_b = plan["last"]
        blk_s = plan["blk_s"]

        cur = {r: (-1, None) for r in range(NRANGE)}  # r -> (chunk id, tile)
        ps = None
        ng = (nb + GRP - 1) // GRP
        for g in range(ng):
            lo_gb = g * GRP
            hi_gb = min(nb, lo_gb + GRP)
            cnt = hi_gb - lo_gb
            # resolve (and issue) gathers for this group's blocks
            resolved = []
            for gb in range(lo_gb, hi_gb):
                r = int(blk_r[gb])
                ck = int(chunk_of[gb])
                if cur[r][0] != ck:
                    (rr_, b0, b1, col) = chunks[ck]
                    assert rr_ == r
                    nblk = b1 - b0
                    t = msgp.tile([128, CG * D], MMDT)
                    m = t[:]
                    out3 = bass.AP(m.tensor, m.offset,
                                   [m.ap[0], [D, nblk], [1, D]])
                    lanes = nblk * 128
                    nc.gpsimd.dma_gather(
                        out3, tables[r][:, :],
                        idx_sb[:, col:col + nblk * 8],
                        lanes, lanes, D, single_packet=False,
                        queue_num=ck % 4)
                    cur[r] = (ck, t)
                t = cur[r][1]
                resolved.append((gb, t, int(stream_pos[gb]) % CG))
            # one-hot for the whole group in a single DVE op
            oh_g = ohp.tile([128, GRP * 256], MMDT)
            dsl = dstl_sb[:, lo_gb:hi_gb].to_broadcast([128, cnt, 256])
            io_ap = iota_sb[:]
            io_b = bass.AP(io_ap.tensor, io_ap.offset,
                           [io_ap.ap[0], [0, cnt], io_ap.ap[1]])
            oh_view = oh_g[:, :cnt * 256]
            oh3 = bass.AP(oh_view.tensor, oh_view.offset,
                          [oh_view.ap[0], [256, cnt], [1, 256]])
            nc.vector.tensor_tensor(out=oh3, in0=dsl, in1=io_b,
                                    op=mybir.AluOpType.is_equal)
            # scatter matmuls
            for j, (gb, t, colk) in enumerate(resolved):
                if first[gb]:
                    ps = psum_st.tile([128, 256], F32)
                nc.tensor.matmul(ps[:], lhsT=t[:, colk * D:(colk + 1) * D],
                                 rhs=oh_g[:, j * 256:(j + 1) * 256],
                                 start=bool(first[gb]), stop=bool(last_b[gb]))
                if last_b[gb]:
                    s = int(blk_s[gb])
                    agg = aggp.tile([128, 256], F32)
                    nc.vector.tensor_copy(agg[:], ps[:])
                    epilogue(s, agg)
    nc.finalize()
    return nc


# ------------------------------------------------------------------- driver

def _pad_rows(a, rows):
    out = np.zeros((rows, a.shape[1]), dtype=a.dtype)
    out[: a.shape[0]] = a
    return out


_cache = {}


def _host_prep(x, edge_index):
    deg, plan, idx, dstloc = _prep_graph(edge_index)
    dis = np.where(deg > 0, 1.0 / np.sqrt(np.maximum(deg, 1.0)),
                   0.0).astype(np.float32)
    xp = _pad_rows(np.asarray(x, np.float32), NPAD)
    ht0 = (dis[:, None] * xp).astype(np.float32)
    # dis laid out [128, NT] per core with dis_pc[c, p, t] = dis[core c, tile t, lane p]
    dis_pc = dis.reshape(NCORES, NT, 128).transpose(0, 2, 1).copy()
    return plan, idx, dstloc, dis_pc, xp, ht0


def kernel(x, edge_index, W1, b1, W2, b2, Wlin, blin):
    x = np.asarray(x, dtype=np.float32)
    W1 = np.asarray(W1, dtype=np.float32)
    b1 = np.asarray(b1, dtype=np.float32)
    W2 = np.asarray(W2, dtype=np.float32)
    b2 = np.asarray(b2, dtype=np.float32)
    Wlin = np.asarray(Wlin, dtype=np.float32)
    blin = np.asarray(blin, dtype=np.float32)

    plan, idx, dstloc, dis_pc, xp, ht0 = _host_prep(x, edge_index)
    iota = np.tile(np.arange(256, dtype=np.float32), (128, 1))
    ident = np.eye(128, dtype=np.float32)
    cores = list(range(NCORES))
    x_pc = xp.reshape(NCORES, NV, D)

    key = tuple(int(k) for k in plan["ks"].reshape(-1))
    if _cache.get("key") != key:
        _cache.clear()
        _cache["key"] = key
        _cache["l1"] = _build_layer(key, last=False)
        _cache["l2"] = _build_layer(key, last=True)

    def tabs(ht):
        return {f"table{r}": ht[r * RV:(r + 1) * RV] for r in range(NRANGE)}

    # ---- launch 1: layer 1
    in1 = [{**tabs(ht0), "idxs": idx[c], "dstl": dstloc[c],
            "res": x_pc[c], "dis": dis_pc[c], "wt": W1.T.copy(),
            "bb": np.tile(b1, (128, 1)), "iota": iota, "ident": ident}
           for c in cores]
    r1 = run_bass_kernel_spmd(_cache["l1"], in1, cores)
    h1_pc = np.stack([r1.results[c]["outh"] for c in cores])
    ht1 = np.concatenate([r1.results[c]["outht"] for c in cores])

    # ---- launch 2: layer 2 + head
    in2 = [{**tabs(ht1), "idxs": idx[c], "dstl": dstloc[c],
            "res": h1_pc[c], "dis": dis_pc[c], "wt": W2.T.copy(),
            "bb": np.tile(b2, (128, 1)), "iota": iota,
            "wl": Wlin.T.copy(), "bl": np.tile(blin, (128, 1)),
            "ident": ident} for c in cores]
    r2 = run_bass_kernel_spmd(_cache["l2"], in2, cores)
    logits = np.concatenate([r2.results[c]["outlg"] for c in cores])
    return logits[:N].astype(np.float32)


# revision 7
# speedup vs baseline: 3.1320x; 2.2469x over previous
"""DiffusionGCN (2-layer GCN + linear head) on 8 Trainium2 NeuronCores.

Strategy (graph/data parallel):
  - Nodes sharded across 8 cores (12544 padded nodes each); edges partitioned
    by destination core, grouped by destination supertile (256 nodes).
  - Symmetric-norm trick: out[v] = dis[v] * sum_{e: dst=v} (dis[src] * h[src]),
    so the source-side scale is folded into the gather table (htilde = dis*h)
    and the dest-side scale is applied after aggregation. W commutes with the
    aggregation and is applied after the segment-sum on the core's own shard.
  - Gathered source features are fetched with bulk `dma_gather` (SWDGE): one
    instruction gathers up to CG*128 rows, amortizing the ~1us fixed SWDGE
    cost that dominates per-row indirect DMA. dma_gather indexes are int16,
    so the node table is split into 4 ranges of 25088 rows; each edge is
    assigned to the gather stream of its source range. Streams are chunked;
    blocks are consumed supertile-major so each supertile's PSUM accumulation
    (one-hot scatter matmuls) closes as soon as its 4 range segments arrive.
  - Segment-sum via one-hot matmuls: per 128-edge block, onehot[e, j] =
    (dstlocal[e] == j) over a 256-wide supertile (single DVE is_equal over a
    group of GRP blocks); PE accumulates msg^T @ onehot into PSUM.
  - 2 SPMD launches: layer 1, layer 2 + classifier head. Host computes deg ->
    dis and htilde0 = dis*x (cheap numpy), and re-shards between launches.
"""

import os
import sys
from contextlib import ExitStack

import numpy as np

for _p in ("/opt/trn_rl_repo", "/root/.axon_site/_ro/trn_rl_repo"):
    if os.path.isdir(_p) and _p not in sys.path:
        sys.path.insert(0, _p)

import concourse.bacc as bacc
import concourse.bass as bass
import concourse.mybir as mybir
import concourse.tile as tile
from concourse.bass_utils import run_bass_kernel_spmd

F32 = mybir.dt.float32
F32R = mybir.dt.float32r
I16 = mybir.dt.int16

N = 100000
E = 1600000
D = 128
H = 128
C = 64
NCORES = 8
NPAD = 100352            # 8 * 12544
NV = NPAD // NCORES      # 12544 nodes per core
NT = NV // 128           # 98 tiles per core
NST = NT // 2            # 49 supertiles (256 nodes) per core
NRANGE = 4               # int16 gather-index ranges
RV = NPAD // NRANGE      # 25088 rows per range table
CG = 16                  # blocks (of 128 edges) per dma_gather chunk
GRP = 8                  # blocks per one-hot DVE build

MMDT = F32R              # dtype of scatter-matmul operands


# ----------------------------------------------------------------- block plan

def _block_plan(ks):
    """Derive per-block/per-chunk metadata from ks[NST, NRANGE] (blocks per
    (supertile, range) segment; identical across cores).

    Consumption (emission) order: supertile-major, range inner, block last.
    Stream order (per range r): supertile-major concatenation of segments.
    Returns dict with:
      nb, blk_r[nb], blk_s[nb], stream_pos[nb], first[nb], last[nb],
      stream_off[NRANGE+1], chunks=[(r, b0, b1, col_off)], chunk_of[nb],
      gb_of_stream[nb]
    """
    ks = np.asarray(ks, np.int64).reshape(NST, NRANGE)
    flat_ks = ks.reshape(-1)
    nb = int(flat_ks.sum())
    blk_s = np.repeat(np.arange(NST), ks.sum(axis=1))
    blk_r = np.repeat(np.tile(np.arange(NRANGE), NST), flat_ks)
    stream_pos = np.zeros(nb, np.int64)
    for r in range(NRANGE):
        sel = blk_r == r
        stream_pos[sel] = np.arange(int(sel.sum()))
    tot_r = [int((blk_r == r).sum()) for r in range(NRANGE)]
    stream_off = np.zeros(NRANGE + 1, np.int64)
    stream_off[1:] = np.cumsum(tot_r)
    gb_of_stream = np.zeros(nb, np.int64)
    gb_of_stream[stream_off[blk_r] + stream_pos] = np.arange(nb)
    # first/last block of each supertile in consumption order
    per_s = ks.sum(axis=1)
    s_start = np.zeros(NST, np.int64)
    s_start[1:] = np.cumsum(per_s)[:-1]
    first = np.zeros(nb, bool)
    last = np.zeros(nb, bool)
    first[s_start] = True
    last[s_start + per_s - 1] = True
    # chunks per range, with idx-const column offsets (8 cols per block)
    chunks = []
    col = 0
    chunk_of = np.zeros(nb, np.int64)
    for r in range(NRANGE):
        nchk = (tot_r[r] + CG - 1) // CG
        for k in range(nchk):
            b0, b1 = k * CG, min((k + 1) * CG, tot_r[r])
            sel = (blk_r == r) & (stream_pos >= b0) & (stream_pos < b1)
            chunk_of[sel] = len(chunks)
            chunks.append((r, b0, b1, col))
            col += (b1 - b0) * 8
    assert col == nb * 8
    return dict(nb=nb, ks=ks, blk_r=blk_r, blk_s=blk_s,
                stream_pos=stream_pos, first=first, last=last,
                stream_off=stream_off, chunks=chunks, chunk_of=chunk_of,
                gb_of_stream=gb_of_stream)


# ----------------------------------------------------------------- host prep

def _prep_graph(edge_index):
    """Partition/sort edges into the (supertile, range) block grid. Returns
    (deg[NPAD] f32, plan, idx[NC,128,nb*8] i16, dstloc[NC,128,nb] f32)."""
    src_all = np.asarray(edge_index[0], dtype=np.int64)
    dst_all = np.asarray(edge_index[1], dtype=np.int64)

    # degree includes the self-loop; the self-loop itself is NOT in the edge
    # lists - its contribution (dis^2 * h_own) comes from the residual tile in
    # the kernel epilogue.
    deg = (np.bincount(dst_all, minlength=NPAD)
           + np.concatenate([np.ones(N), np.zeros(NPAD - N)])).astype(np.float32)

    core = dst_all // NV
    stl = (dst_all % NV) // 256
    rr = src_all // RV

    counts = np.zeros((NCORES, NST, NRANGE), np.int64)
    np.add.at(counts, (core, stl, rr), 1)
    ks = np.ceil(counts.max(axis=0) / 128).astype(np.int64)
    ks = np.maximum(ks, 1)
    plan = _block_plan(ks)
    nb = plan["nb"]
    flat_ks = ks.reshape(-1)
    gb_base = np.zeros(NST * NRANGE, np.int64)
    gb_base[1:] = np.cumsum(flat_ks)[:-1]
    gb_base = gb_base.reshape(NST, NRANGE)

    lane_src = np.full((NCORES, nb, 128), -1, np.int64)
    lane_dst = np.full((NCORES, nb, 128), -1.0, np.float32)
    for c in range(NCORES):
        m = core == c
        s_c = src_all[m]
        d_c = dst_all[m]
        st_c = stl[m]
        r_c = rr[m]
        # sort by (supertile, range, src) - src order improves HBM locality
        order = np.lexsort((s_c, r_c, st_c))
        s_c, d_c, st_c, r_c = s_c[order], d_c[order], st_c[order], r_c[order]
        loc = s_c - r_c * RV
        key = st_c * NRANGE + r_c
        seg_starts = np.searchsorted(key, np.arange(NST * NRANGE))
        j = np.arange(len(s_c)) - seg_starts[key]
        gb = gb_base[st_c, r_c] + j // 128
        lane = j % 128
        lane_src[c, gb, lane] = loc
        lane_dst[c, gb, lane] = (d_c % NV) - st_c * 256.0
        # pad unfilled lanes with the segment's last real index (keeps HBM
        # row locality); dstloc stays -1 so the one-hot kills them.
        segc = counts[c].reshape(-1)
        pad = np.zeros(NST * NRANGE, np.int64)
        nz = segc > 0
        pad[nz] = loc[(seg_starts + segc - 1)[nz]]
        blk_pad = np.repeat(pad, flat_ks)
        msk = lane_src[c] < 0
        lane_src[c][msk] = np.broadcast_to(blk_pad[:, None], (nb, 128))[msk]

    # idx const [NC, 128, nb*8]: per-chunk 16-lane wrap, replicated x8
    lane_src_stream = lane_src[:, plan["gb_of_stream"], :]
    idx = np.zeros((NCORES, 128, nb * 8), np.int16)
    so = plan["stream_off"]
    for (r, b0, b1, col) in plan["chunks"]:
        cb = b1 - b0
        sl = lane_src_stream[:, so[r] + b0:so[r] + b1, :]      # [NC, cb, 128]
        flat = sl.reshape(NCORES, cb * 128)
        wrapped = flat.reshape(NCORES, cb * 8, 16).transpose(0, 2, 1)
        idx[:, :, col:col + cb * 8] = np.tile(wrapped, (1, 8, 1)).astype(np.int16)

    dstloc = lane_dst.transpose(0, 2, 1).copy()                # [NC, 128, nb]
    return deg, plan, idx, dstloc


# ------------------------------------------------------------ kernel builder

def _build_layer(ks_key, last, msg_bufs=10, oh_bufs=4):
    """One GCN layer. last=False: outputs h (relu(conv)+res) and htilde=dis*h.
    last=True: second layer fused with the classifier head, outputs logits.

    Self-loop contribution is not gathered: z = (agg + dis^2*res) @ W.T via a
    second accumulating matmul off the transposed scaled residual."""
    plan = _block_plan(np.asarray(ks_key, np.int64).reshape(NST, NRANGE))
    nb = plan["nb"]
    nc = bacc.Bacc("TRN2", num_swdge_queues=4)
    tables = [nc.dram_tensor(f"table{r}", [RV, D], MMDT, kind="ExternalInput")
              for r in range(NRANGE)]
    idxs = nc.dram_tensor("idxs", [128, nb * 8], I16, kind="ExternalInput")
    dstl = nc.dram_tensor("dstl", [128, nb], F32, kind="ExternalInput")
    res_in = nc.dram_tensor("res", [NV, D], F32, kind="ExternalInput")
    dis_in = nc.dram_tensor("dis", [128, NT], F32, kind="ExternalInput")
    wt_in = nc.dram_tensor("wt", [D, H], F32, kind="ExternalInput")    # W.T
    bb_in = nc.dram_tensor("bb", [128, H], F32, kind="ExternalInput")  # bias bcast
    iota_in = nc.dram_tensor("iota", [128, 256], F32, kind="ExternalInput")
    id_in = nc.dram_tensor("ident", [128, 128], F32, kind="ExternalInput")
    if last:
        wl_in = nc.dram_tensor("wl", [H, C], F32, kind="ExternalInput")   # Wlin.T
        bl_in = nc.dram_tensor("bl", [128, C], F32, kind="ExternalInput")
        out_lg = nc.dram_tensor("outlg", [NV, C], F32, kind="ExternalOutput")
    else:
        out_h = nc.dram_tensor("outh", [NV, D], F32, kind="ExternalOutput")
        out_ht = nc.dram_tensor("outht", [NV, D], F32, kind="ExternalOutput")

    with tile.TileContext(nc) as tc, ExitStack() as ctx:
        const = ctx.enter_context(tc.tile_pool(name="const", bufs=1))
        msgp = ctx.enter_context(tc.tile_pool(name="msg", bufs=msg_bufs))
        ohp = ctx.enter_context(tc.tile_pool(name="oh", bufs=oh_bufs))
        aggp = ctx.enter_context(tc.tile_pool(name="agg", bufs=3))
        ep = ctx.enter_context(tc.tile_pool(name="ep", bufs=6))
        psum_st = ctx.enter_context(tc.tile_pool(name="pst", bufs=2, space="PSUM"))
        psum_z = ctx.enter_context(tc.tile_pool(name="pz", bufs=2, space="PSUM"))
        psum_t = ctx.enter_context(tc.tile_pool(name="ptr", bufs=2, space="PSUM"))
        if last:
            psum_l = ctx.enter_context(tc.tile_pool(name="plg", bufs=2, space="PSUM"))

        idx_sb = const.tile([128, nb * 8], I16)
        nc.sync.dma_start(idx_sb[:], idxs[:])
        dstl_sb = const.tile([128, nb], F32)
        nc.sync.dma_start(dstl_sb[:], dstl[:])
        iota_sb = const.tile([128, 256], F32)
        nc.sync.dma_start(iota_sb[:], iota_in[:])
        dis_sb = const.tile([128, NT], F32)
        nc.sync.dma_start(dis_sb[:], dis_in[:])
        wt_sb = const.tile([D, H], F32)
        nc.sync.dma_start(wt_sb[:], wt_in[:])
        bb_sb = const.tile([128, H], F32)
        nc.sync.dma_start(bb_sb[:], bb_in[:])
        id_sb = const.tile([128, 128], F32)
        nc.sync.dma_start(id_sb[:], id_in[:])
        if last:
            wl_sb = const.tile([H, C], F32)
            nc.sync.dma_start(wl_sb[:], wl_in[:])
            bl_sb = const.tile([128, C], F32)
            nc.sync.dma_start(bl_sb[:], bl_in[:])

        def epilogue(s, agg):
            for t2 in range(2):
                t = 2 * s + t2
                res_t = ep.tile([128, D], F32)
                nc.sync.dma_start(res_t[:], res_in[t * 128:(t + 1) * 128, :])
                sres = ep.tile([128, D], F32)
                nc.scalar.activation(sres[:], res_t[:],
                                     mybir.ActivationFunctionType.Identity,
                                     scale=dis_sb[:, t:t + 1])
                tp = psum_t.tile([128, D], F32)
                nc.tensor.transpose(tp[:], sres[:], id_sb[:])
                sresT = ep.tile([128, D], F32)
                nc.vector.tensor_copy(sresT[:], tp[:])
                z = psum_z.tile([128, H], F32)
                nc.tensor.matmul(z[:], lhsT=agg[:, t2 * 128:(t2 + 1) * 128],
                                 rhs=wt_sb[:], start=True, stop=False)
                nc.tensor.matmul(z[:], lhsT=sresT[:], rhs=wt_sb[:],
                                 start=False, stop=True)
                zs = ep.tile([128, H], F32)
                nc.scalar.activation(zs[:], z[:],
                                     mybir.ActivationFunctionType.Identity,
                                     scale=dis_sb[:, t:t + 1])
                zb = ep.tile([128, H], F32)
                nc.vector.tensor_tensor(out=zb[:], in0=zs[:], in1=bb_sb[:],
                                        op=mybir.AluOpType.add)
                hr = ep.tile([128, H], F32)
                nc.scalar.activation(hr[:], zb[:], mybir.ActivationFunctionType.Relu)
                h = ep.tile([128, H], F32)
                nc.vector.tensor_tensor(out=h[:], in0=hr[:], in1=res_t[:],
                                        op=mybir.AluOpType.add)
                if not last:
                    nc.scalar.dma_start(out_h[t * 128:(t + 1) * 128, :], h[:])
                    ht = ep.tile([128, H], F32)
                    nc.scalar.activation(ht[:], h[:],
                                         mybir.ActivationFunctionType.Identity,
                                         scale=dis_sb[:, t:t + 1])
                    nc.scalar.dma_start(out_ht[t * 128:(t + 1) * 128, :], ht[:])
                else:
                    tp2 = psum_t.tile([128, H], F32, tag="tp")
                    nc.tensor.transpose(tp2[:], h[:], id_sb[:])
                    hT = ep.tile([128, H], F32)
                    nc.vector.tensor_copy(hT[:], tp2[:])
                    lg = psum_l.tile([128, C], F32)
                    nc.tensor.matmul(lg[:], lhsT=hT[:], rhs=wl_sb[:],
                                     start=True, stop=True)
                    lo = ep.tile([128, C], F32)
                    nc.vector.tensor_tensor(out=lo[:], in0=lg[:], in1=bl_sb[:],
                                            op=mybir.AluOpType.add)
                    nc.scalar.dma_start(out_lg[t * 128:(t + 1) * 128, :], lo[:])

        chunks = plan["chunks"]
        blk_r = plan["blk_r"]
        chunk_of = plan["chunk_of"]
        stream_pos = plan["stream_pos"]
        first = plan["first"]
        last_b = plan["last"]
        blk_s = plan["blk_s"]

        cur = {r: (-1, None) for r in range(NRANGE)}  # r -> (chunk id, tile)
        qctr = [0]
        ps = None
        ng = (nb + GRP - 1) // GRP
        for g in range(ng):
            lo_gb = g * GRP
            hi_gb = min(nb, lo_gb + GRP)
            cnt = hi_gb - lo_gb
            # resolve (and issue) gathers for this group's blocks
            resolved = []
            for gb in range(lo_gb, hi_gb):
                r = int(blk_r[gb])
                ck = int(chunk_of[gb])
                if cur[r][0] != ck:
                    (rr_, b0, b1, col) = chunks[ck]
                    assert rr_ == r
                    nblk = b1 - b0
                    t = msgp.tile([128, CG * D], MMDT)
                    m = t[:]
                    out3 = bass.AP(m.tensor, m.offset,
                                   [m.ap[0], [D, nblk], [1, D]])
                    lanes = nblk * 128
                    nc.gpsimd.dma_gather(
                        out3, tables[r][:, :],
                        idx_sb[:, col:col + nblk * 8],
                        lanes, lanes, D, single_packet=False,
                        queue_num=qctr[0] % 4)
                    qctr[0] += 1
                    cur[r] = (ck, t)
                t = cur[r][1]
                resolved.append((gb, t, int(stream_pos[gb]) % CG))
            # one-hot for the whole group in a single DVE op
            oh_g = ohp.tile([128, GRP * 256], MMDT)
            dsl = dstl_sb[:, lo_gb:hi_gb].to_broadcast([128, cnt, 256])
            io_ap = iota_sb[:]
            io_b = bass.AP(io_ap.tensor, io_ap.offset,
                           [io_ap.ap[0], [0, cnt], io_ap.ap[1]])
            oh_view = oh_g[:, :cnt * 256]
            oh3 = bass.AP(oh_view.tensor, oh_view.offset,
                          [oh_view.ap[0], [256, cnt], [1, 256]])
            nc.vector.tensor_tensor(out=oh3, in0=dsl, in1=io_b,
                                    op=mybir.AluOpType.is_equal)
            # scatter matmuls
            for j, (gb, t, colk) in enumerate(resolved):
                if first[gb]:
                    ps = psum_st.tile([128, 256], F32)
                nc.tensor.matmul(ps[:], lhsT=t[:, colk * D:(colk + 1) * D],
                                 rhs=oh_g[:, j * 256:(j + 1) * 256],
                                 start=bool(first[gb]), stop=bool(last_b[gb]))
                if last_b[gb]:
                    s = int(blk_s[gb])
                    agg = aggp.tile([128, 256], F32)
                    nc.vector.tensor_copy(agg[:], ps[:])
                    epilogue(s, agg)
    nc.finalize()
    return nc


# ------------------------------------------------------------------- driver

def _pad_rows(a, rows):
    out = np.zeros((rows, a.shape[1]), dtype=a.dtype)
    out[: a.shape[0]] = a
    return out


_cache = {}


def _host_prep(x, edge_index):
    deg, plan, idx, dstloc = _prep_graph(edge_index)
    dis = np.where(deg > 0, 1.0 / np.sqrt(np.maximum(deg, 1.0)),
                   0.0).astype(np.float32)
    xp = _pad_rows(np.asarray(x, np.float32), NPAD)
    ht0 = (dis[:, None] * xp).astype(np.float32)
    # dis laid out [128, NT] per core with dis_pc[c, p, t] = dis[core c, tile t, lane p]
    dis_pc = dis.reshape(NCORES, NT, 128).transpose(0, 2, 1).copy()
    return plan, idx, dstloc, dis_pc, xp, ht0


def kernel(x, edge_index, W1, b1, W2, b2, Wlin, blin):
    x = np.asarray(x, dtype=np.float32)
    W1 = np.asarray(W1, dtype=np.float32)
    b1 = np.asarray(b1, dtype=np.float32)
    W2 = np.asarray(W2, dtype=np.float32)
    b2 = np.asarray(b2, dtype=np.float32)
    Wlin = np.asarray(Wlin, dtype=np.float32)
    blin = np.asarray(blin, dtype=np.float32)

    plan, idx, dstloc, dis_pc, xp, ht0 = _host_prep(x, edge_index)
    iota = np.tile(np.arange(256, dtype=np.float32), (128, 1))
    ident = np.eye(128, dtype=np.float32)
    cores = list(range(NCORES))
    x_pc = xp.reshape(NCORES, NV, D)

    key = tuple(int(k) for k in plan["ks"].reshape(-1))
    if _cache.get("key") != key:
        _cache.clear()
        _cache["key"] = key
        _cache["l1"] = _build_layer(key, last=False)
        _cache["l2"] = _build_layer(key, last=True)

    def tabs(ht):
        return {f"table{r}": ht[r * RV:(r + 1) * RV] for r in range(NRANGE)}

    # ---- launch 1: layer 1
    in1 = [{**tabs(ht0), "idxs": idx[c], "dstl": dstloc[c],
            "res": x_pc[c], "dis": dis_pc[c], "wt": W1.T.copy(),
            "bb": np.tile(b1, (128, 1)), "iota": iota, "ident": ident}
           for c in cores]
    r1 = run_bass_kernel_spmd(_cache["l1"], in1, cores)
    h1_pc = np.stack([r1.results[c]["outh"] for c in cores])
    ht1 = np.concatenate([r1.results[c]["outht"] for c in cores])

    # ---- launch 2: layer 2 + head
    in2 = [{**tabs(ht1), "idxs": idx[c], "dstl": dstloc[c],
            "res": h1_pc[c], "dis": dis_pc[c], "wt": W2.T.copy(),
            "bb": np.tile(b2, (128, 1)), "iota": iota,
            "wl": Wlin.T.copy(), "bl": np.tile(blin, (128, 1)),
            "ident": ident} for c in cores]
    r2 = run_bass_kernel_spmd(_cache["l2"], in2, cores)
    logits = np.concatenate([r2.results[c]["outlg"] for c in cores])
    return logits[:N].astype(np.float32)
